# revision 32
# baseline (speedup 1.0000x reference)
"""Trainium2 Bass kernel for nn_AbsorberPathAggregator.

Strategy (sharding_hint): shard paths (P=8000) across 8 cores, replicate
params and e_feat. Paths are sorted by batch and padded per-batch to tiles of
128; a batch's tiles are assigned round-robin to distinct cores so each
(core, batch) pair owns at most one tile. Each tile produces a [128, 42]
"slot" block (40 cols of folded agg + Sgg col + norm col) that is
indirect-DMA-scattered into a per-core DRAM accumulator indexed by batch
(data-driven, so one SPMD program serves all cores). One AllReduce combines
cores; each core then normalizes + out-MLPs only its own 2 batches (indirect
gather/scatter make the batch choice data-driven) and the host assembles the
slices.

Layout highlights:
  - all geometry / cutoff / gating math runs as a single wide prepass over
    all T tiles at once ([*, T*128] ops) to amortize per-instruction cost;
  - the (path, e) pair-MLP folds e two-wide onto 128 partitions and runs
    through float32r matmuls (full-rate fp32) with the u+v broadcast-add on
    the otherwise-idle GPSIMD engine;
  - contribution mul+reduce on DVE writes per-tile slot columns directly.
"""

import os

import numpy as np

import concourse.bacc as bacc
import concourse.bass as bass
import concourse.mybir as mybir
import concourse.tile as tile
from concourse.bass_utils import run_bass_kernel_spmd

F32 = mybir.dt.float32
F32R = mybir.dt.float32r
BF16 = mybir.dt.bfloat16
I32 = mybir.dt.int32

NCORES = 8
B = 16
BL = B // NCORES   # local batches per core in the endgame
NE = 80
S = 64             # scatter dim
EP = NE // 2       # folded e-pairs
F = 128            # paths per tile
EPC = 8            # e-pairs per psum chunk (8*128 = 1024 cols = 2 psum banks)
NSLOT = 17         # 16 batches + 1 trash
SLOTC = 42         # 40 agg cols + Sgg col + norm col
ATOM = 128
RBF = 32
CUT = 5.0
RBF_SCALE = (RBF - 1) / CUT      # 1/width = 31/5
PI = float(np.pi)

_NC_CACHE = {}


def _bc_mid(ap, n):
    """[P, F] -> [P, n, F] with 0-step middle dim."""
    l = [list(x) for x in ap.ap]
    assert len(l) == 2, l
    return bass.AP(ap.tensor, ap.offset, [l[0], [0, n], l[1]])


def _bc_last(ap, n):
    """[...dims] -> [...dims, n] with 0-step last dim."""
    l = [list(x) for x in ap.ap]
    return bass.AP(ap.tensor, ap.offset, l + [[0, n]])


def build_nc(T: int, TH: int) -> bass.Bass:
    nc = bacc.Bacc("TRN2", target_bir_lowering=False, debug=False,
                   num_devices=NCORES)
    AF = mybir.ActivationFunctionType
    ALU = mybir.AluOpType
    W = T * F   # wide prepass width

    # ---- per-core inputs (wide layout: [rows, T*F])
    hjT_d = nc.dram_tensor("hjT", [ATOM, W], F32R, kind="ExternalInput")
    hkT_d = nc.dram_tensor("hkT", [ATOM, W], F32R, kind="ExternalInput")
    ejk2_d = nc.dram_tensor("ejk2", [128, W], F32R, kind="ExternalInput")
    r96_d = nc.dram_tensor("r96", [96, W], F32, kind="ExternalInput")
    cos1_d = nc.dram_tensor("cos1", [1, W], F32R, kind="ExternalInput")
    oidx_d = nc.dram_tensor("oidx", [BL, NE, 1], I32, kind="ExternalInput")
    gidx_d = nc.dram_tensor("gidx", [BL, 128, 1], I32, kind="ExternalInput")
    sidx_d = nc.dram_tensor("sidx", [T, 128, 1], I32, kind="ExternalInput")
    # ---- replicated params
    gw1a_d = nc.dram_tensor("gw1a", [128, 128], F32R, kind="ExternalInput")
    gw1b_d = nc.dram_tensor("gw1b", [128, 128], F32R, kind="ExternalInput")
    gw1c_d = nc.dram_tensor("gw1c", [97, 128], F32R, kind="ExternalInput")
    gb1_d = nc.dram_tensor("gb1", [128, 1], F32, kind="ExternalInput")
    gw2_d = nc.dram_tensor("gw2", [128, 128], F32R, kind="ExternalInput")
    gb2_d = nc.dram_tensor("gb2", [128, 1], F32, kind="ExternalInput")
    gw3_d = nc.dram_tensor("gw3", [128, 64], F32R, kind="ExternalInput")
    gb3_d = nc.dram_tensor("gb3", [64, 1], F32, kind="ExternalInput")
    w1ab2_d = nc.dram_tensor("w1ab2", [128, 128], F32R, kind="ExternalInput")
    b1c_d = nc.dram_tensor("b1c", [64, 1], F32, kind="ExternalInput")
    w2bd_d = nc.dram_tensor("w2bd", [128, 128], F32R, kind="ExternalInput")
    b2c2_d = nc.dram_tensor("b2c2", [128, 1], F32, kind="ExternalInput")
    w3bd_d = nc.dram_tensor("w3bd", [128, 128], F32R, kind="ExternalInput")
    b3c2_d = nc.dram_tensor("b3c2", [128, 1], F32, kind="ExternalInput")
    efev_d = nc.dram_tensor("efev", [32, EP], F32, kind="ExternalInput")
    efod_d = nc.dram_tensor("efod", [32, EP], F32, kind="ExternalInput")
    w1c_d = nc.dram_tensor("w1c", [32, 64], F32, kind="ExternalInput")
    ow1_d = nc.dram_tensor("ow1", [64, 128], F32, kind="ExternalInput")
    ob1_d = nc.dram_tensor("ob1", [128, 1], F32, kind="ExternalInput")
    ow2_d = nc.dram_tensor("ow2", [128, 64], F32, kind="ExternalInput")
    ob2_d = nc.dram_tensor("ob2", [64, 1], F32, kind="ExternalInput")
    rb96_d = nc.dram_tensor("rb96", [96, 1], F32, kind="ExternalInput")
    ones128_d = nc.dram_tensor("ones128", [1, 128], F32, kind="ExternalInput")
    id128_d = nc.dram_tensor("id128", [128, 128], F32, kind="ExternalInput")
    # ---- output + collective bounce
    out_d = nc.dram_tensor("out", [B * NE, S], F32, kind="ExternalOutput")
    aggin_d = nc.dram_tensor("aggin", [NSLOT * 128, SLOTC], F32)
    aggout_d = nc.dram_tensor("aggout", [B * 128, SLOTC], F32,
                              addr_space="Shared")

    with tile.TileContext(nc) as tc:
        with tc.tile_pool(name="const", bufs=1) as cp:
            def cload(dram, shape, dt=F32):
                t = cp.tile(shape, dt, tag=dram.name)
                nc.sync.dma_start(t[:], dram[:])
                return t

            gw1a = cload(gw1a_d, [128, 128], F32R)
            gw1b = cload(gw1b_d, [128, 128], F32R)
            gw1c = cload(gw1c_d, [97, 128], F32R)
            gb1 = cload(gb1_d, [128, 1])
            gw2 = cload(gw2_d, [128, 128], F32R)
            gb2 = cload(gb2_d, [128, 1])
            gw3 = cload(gw3_d, [128, 64], F32R)
            gb3 = cload(gb3_d, [64, 1])
            w1ab2 = cload(w1ab2_d, [128, 128], F32R)
            b1c = cload(b1c_d, [64, 1])
            w2bd = cload(w2bd_d, [128, 128], F32R)
            b2c2 = cload(b2c2_d, [128, 1])
            w3bd = cload(w3bd_d, [128, 128], F32R)
            b3c2 = cload(b3c2_d, [128, 1])
            efev = cload(efev_d, [32, EP])
            efod = cload(efod_d, [32, EP])
            w1c = cload(w1c_d, [32, 64])
            ow1 = cload(ow1_d, [64, 128])
            ob1 = cload(ob1_d, [128, 1])
            ow2 = cload(ow2_d, [128, 64])
            ob2 = cload(ob2_d, [64, 1])
            rb96 = cload(rb96_d, [96, 1])
            ones128 = cload(ones128_d, [1, 128])
            id128 = cload(id128_d, [128, 128])

            pih = cp.tile([128, 1], F32, tag="pih")
            nc.vector.memset(pih[:], PI / 2)

            # zero-fill the scatter accumulator
            zslot = cp.tile([128, SLOTC], F32, tag="zslot")
            nc.vector.memset(zslot[:], 0.0)
            for s in range(NSLOT):
                nc.sync.dma_start(
                    aggin_d[s * 128:(s + 1) * 128, :], zslot[:])

            # v2 [128, EP]: v2[0:64, e'] = v(2e')+b1, v2[64:, e'] = v(2e'+1)+b1
            v2 = cp.tile([128, EP], F32, tag="v2")

            with (
                tc.tile_pool(name="pre", bufs=1) as pre,
                tc.tile_pool(name="wk", bufs=3) as wk,
                tc.tile_pool(name="io", bufs=3) as io,
            ):
              with tc.tile_pool(name="psp", bufs=1, space="PSUM") as psp:
                vps = psp.tile([64, EP], F32, tag="pa")
                nc.tensor.matmul(vps[:], w1c[:], efev[:], start=True, stop=True)
                nc.scalar.activation(v2[0:64, :], vps[:], AF.Copy)
                nc.vector.tensor_scalar_add(v2[0:64, :], v2[0:64, :], b1c[:])
                vps2 = psp.tile([64, EP], F32, tag="pb")
                nc.tensor.matmul(vps2[:], w1c[:], efod[:], start=True, stop=True)
                nc.scalar.activation(v2[64:128, :], vps2[:], AF.Copy)
                nc.vector.tensor_scalar_add(v2[64:128, :], v2[64:128, :],
                                            b1c[:])

                # ---- wide input loads, chunked across DMA queues
                hjA = pre.tile([ATOM, W], F32R, tag="hjA")
                hkA = pre.tile([ATOM, W], F32R, tag="hkA")
                ejA = pre.tile([128, W], F32R, tag="ejA")
                r96A = pre.tile([96, W], F32, tag="r96A")
                NQ = 4
                for q in range(NQ):
                    k0 = q * W // NQ
                    k1 = (q + 1) * W // NQ
                    nc.sync.dma_start(ejA[:, k0:k1], ejk2_d[:, k0:k1])
                    nc.sync.dma_start(r96A[:, k0:k1], r96_d[:, k0:k1])
                    nc.sync.dma_start(hjA[:, k0:k1], hjT_d[:, k0:k1])
                    nc.sync.dma_start(hkA[:, k0:k1], hkT_d[:, k0:k1])

                def wchunks():
                    out = []
                    k = 0
                    while k < W:
                        out.append((k, min(k + 512, W)))
                        k += 512
                    return out

                # ---- u2A = blockdiag(w1ab,w1ab)^T @ [ej;ek;ej;ek], all tiles
                u2pA = psp.tile([128, W], F32, tag="pa")
                for k, ke in wchunks():
                    nc.tensor.matmul(u2pA[:, k:ke], w1ab2[:], ejA[:, k:ke],
                                     start=True, stop=True)
                u2A = pre.tile([128, W], F32, tag="u2A")
                nc.scalar.copy(u2A[:], u2pA[:])

                # ---- geom MLP wide
                grbfA = pre.tile([97, W], F32R, tag="grbfA")
                rc96A = pre.tile([96, W], F32, tag="rc96A")
                nc.vector.tensor_scalar_min(rc96A[:], r96A[:], CUT)
                sq96A = pre.tile([96, W], F32, tag="sq96A")
                nc.scalar.activation(sq96A[:], rc96A[:], AF.Square,
                                     bias=rb96[:], scale=RBF_SCALE)
                nc.scalar.activation(grbfA[0:96, :], sq96A[:], AF.Exp,
                                     scale=-0.5)
                nc.sync.dma_start(grbfA[96:97, :], cos1_d[:])

                gpA = psp.tile([128, W], F32, tag="pb")
                for k, ke in wchunks():
                    nc.tensor.matmul(gpA[:, k:ke], gw1a[:], hjA[:, k:ke],
                                     start=True, stop=False)
                    nc.tensor.matmul(gpA[:, k:ke], gw1b[:], hkA[:, k:ke],
                                     start=False, stop=False)
                    nc.tensor.matmul(gpA[:, k:ke], gw1c[:], grbfA[:, k:ke],
                                     start=False, stop=True)
                h1gA = pre.tile([128, W], F32R, tag="h1gA")
                nc.scalar.activation(h1gA[:], gpA[:], AF.Silu, bias=gb1[:])
                gp2A = psp.tile([128, W], F32, tag="pa")
                for k, ke in wchunks():
                    nc.tensor.matmul(gp2A[:, k:ke], gw2[:], h1gA[:, k:ke],
                                     start=True, stop=True)
                h2gA = pre.tile([128, W], F32R, tag="h2gA")
                nc.scalar.activation(h2gA[:], gp2A[:], AF.Silu, bias=gb2[:])
                g3pA = psp.tile([64, W], F32, tag="pb")
                for k, ke in wchunks():
                    nc.tensor.matmul(g3pA[:, k:ke], gw3[:], h2gA[:, k:ke],
                                     start=True, stop=True)

                # ---- cutoff weights wide: cutoff(r) = cos^2(pi*r/10)*(r<5)
                snA = pre.tile([96, W], F32, tag="snA")
                nc.scalar.activation(snA[:], rc96A[:], AF.Sin,
                                     bias=pih[0:96, :], scale=PI / (2 * CUT))
                cwmA = pre.tile([96, W], F32, tag="cwmA")
                nc.vector.tensor_tensor(cwmA[:], snA[:], snA[:], op=ALU.mult)
                mkA = pre.tile([96, W], F32, tag="mkA")
                nc.vector.tensor_scalar(mkA[:], r96A[:], CUT, None,
                                        op0=ALU.is_lt)
                nc.vector.tensor_tensor(cwmA[:], cwmA[:], mkA[:], op=ALU.mult)
                cwBA = pre.tile([32, W], F32, tag="cwBA")
                nc.vector.tensor_copy(cwBA[:], cwmA[32:64, :])
                cwDA = pre.tile([32, W], F32, tag="cwDA")
                nc.vector.tensor_copy(cwDA[:], cwmA[64:96, :])
                cwCA = pre.tile([64, W], F32, tag="cwCA")
                nc.vector.tensor_tensor(cwCA[0:32, :], cwmA[0:32, :], cwBA[:],
                                        op=ALU.mult)
                nc.vector.tensor_tensor(cwCA[0:32, :], cwCA[0:32, :], cwDA[:],
                                        op=ALU.mult)
                nc.vector.tensor_copy(cwCA[32:64, :], cwCA[0:32, :])

                # ---- gg2A = [cw*(g3+b3); same]
                t3A = pre.tile([64, W], F32, tag="t3A")
                nc.vector.tensor_scalar_add(t3A[:], g3pA[:], gb3[:])
                gg2A = pre.tile([128, W], F32, tag="gg2A")
                nc.vector.tensor_tensor(gg2A[0:64, :], t3A[:], cwCA[:],
                                        op=ALU.mult)
                nc.gpsimd.tensor_copy(gg2A[64:128, :], gg2A[0:64, :])

              with tc.tile_pool(name="psh", bufs=2, space="PSUM") as psh:
                # ---- per-tile elem MLP + slot scatter
                for t in range(T):
                    ts = slice(t * F, (t + 1) * F)
                    sx = io.tile([128, 1], I32, tag="sx")
                    nc.sync.dma_start(sx[:], sidx_d[t])
                    slot = wk.tile([128, SLOTC], F32, tag="slot")
                    nc.vector.tensor_reduce(slot[:, 40:41], gg2A[:, ts],
                                            axis=mybir.AxisListType.X,
                                            op=ALU.add)
                    nc.vector.memset(slot[:, 41:42], 0.0)
                    nc.vector.tensor_reduce(slot[0:1, 41:42], cwCA[0:1, ts],
                                            axis=mybir.AxisListType.X,
                                            op=ALU.add)

                    for c in range(EP // EPC):
                        h1pre = wk.tile([128, EPC, F], F32, tag="h1pre")
                        nc.gpsimd.tensor_tensor(
                            h1pre[:], _bc_mid(u2A[:, ts], EPC),
                            _bc_last(v2[:, c * EPC:(c + 1) * EPC], F),
                            op=ALU.add)
                        h1 = wk.tile([128, EPC, F], F32R, tag="h1s")
                        nc.scalar.activation(h1[:], h1pre[:], AF.Silu)
                        h2p = psh.tile([128, EPC, F], F32, tag="hpa")
                        for hf in range(2):
                            sl = slice(hf * 4, hf * 4 + 4)
                            nc.tensor.matmul(h2p[:, sl, :], w2bd[:],
                                             h1[:, sl, :], start=True,
                                             stop=True)
                        h2 = wk.tile([128, EPC, F], F32R, tag="h2s")
                        nc.scalar.activation(h2[:], h2p[:], AF.Silu,
                                             bias=b2c2[:])
                        h3p = psh.tile([128, EPC, F], F32, tag="hpb")
                        for hf in range(2):
                            sl = slice(hf * 4, hf * 4 + 4)
                            nc.tensor.matmul(h3p[:, sl, :], w3bd[:],
                                             h2[:, sl, :], start=True,
                                             stop=True)
                        ctb = wk.tile([128, EPC, F], F32, tag="ctb")
                        nc.vector.tensor_tensor(ctb[:], h3p[:],
                                                _bc_mid(gg2A[:, ts], EPC),
                                                op=ALU.mult)
                        nc.vector.tensor_reduce(
                            slot[:, c * EPC:(c + 1) * EPC], ctb[:],
                            axis=mybir.AxisListType.X, op=ALU.add)

                    nc.gpsimd.indirect_dma_start(
                        out=aggin_d[:, :],
                        out_offset=bass.IndirectOffsetOnAxis(ap=sx[:, 0:1],
                                                             axis=0),
                        in_=slot[:, :],
                        in_offset=None,
                    )
                    if t == TH - 1:
                        # batches 0..7 are complete on every core: reduce
                        # them while the high-batch tiles still compute
                        nc.gpsimd.collective_compute(
                            "AllReduce",
                            mybir.AluOpType.add,
                            replica_groups=[list(range(NCORES))],
                            ins=[aggin_d[0:(B // 2) * 128, :]],
                            outs=[aggout_d[0:(B // 2) * 128, :]],
                        )

            # ---- AllReduce over the remaining batch slots
            nc.gpsimd.collective_compute(
                "AllReduce",
                mybir.AluOpType.add,
                replica_groups=[list(range(NCORES))],
                ins=[aggin_d[(B // 2) * 128:B * 128, :]],
                outs=[aggout_d[(B // 2) * 128:B * 128, :]],
            )

            # ---- endgame: each core normalizes + out-MLPs its BL batches
            with (
                tc.tile_pool(name="eg", bufs=1) as eg,
                tc.tile_pool(name="egp", bufs=2, space="PSUM") as egp,
            ):
                gxt = eg.tile([128, BL], I32, tag="gxt")
                oxt = eg.tile([NE, BL], I32, tag="oxt")
                agg2 = eg.tile([128, BL, SLOTC], F32, tag="agg2")
                for bl in range(BL):
                    nc.sync.dma_start(gxt[:, bl:bl + 1], gidx_d[bl])
                    nc.sync.dma_start(oxt[:, bl:bl + 1], oidx_d[bl])
                    nc.gpsimd.indirect_dma_start(
                        out=agg2[:, bl, :],
                        out_offset=None,
                        in_=aggout_d[:, :],
                        in_offset=bass.IndirectOffsetOnAxis(
                            ap=gxt[:, bl:bl + 1], axis=0),
                    )
                norm2 = eg.tile([1, BL], F32, tag="norm2")
                nc.vector.tensor_scalar_max(norm2[:], agg2[0:1, :, 41], 1e-8)
                rn = eg.tile([1, BL], F32, tag="rn")
                nc.vector.reciprocal(rn[:], norm2[:])
                rnp = egp.tile([128, BL], F32, tag="rnp")
                nc.tensor.matmul(rnp[:], ones128[:], rn[:], start=True,
                                 stop=True)

                sgb = eg.tile([128, BL, EP], F32, tag="sgb")
                nc.vector.tensor_scalar(sgb[:], _bc_last(agg2[:, :, 40], EP),
                                        b3c2[:], None, op0=ALU.mult)
                t2 = eg.tile([128, BL, EP], F32, tag="t2")
                nc.vector.tensor_tensor(t2[:], sgb[:], agg2[:, :, 0:40],
                                        op=ALU.add)
                agn = eg.tile([128, BL, EP], F32, tag="agn")
                nc.vector.tensor_tensor(agn[:], t2[:], _bc_last(rnp[:, :], EP),
                                        op=ALU.mult)
                unf = eg.tile([64, BL, NE], F32, tag="unf")
                nc.vector.tensor_copy(unf[:, :, 0::2], agn[0:64, :, :])
                nc.vector.tensor_copy(unf[:, :, 1::2], agn[64:128, :, :])

                NCOL = BL * NE
                unf_f = unf[:, :, :].rearrange("p a b -> p (a b)")
                hop = egp.tile([128, NCOL], F32, tag="hop")
                nc.tensor.matmul(hop[:], ow1[:], unf_f[:], start=True,
                                 stop=True)
                ho = eg.tile([128, BL, NE], F32, tag="ho")
                ho_f = ho[:, :, :].rearrange("p a b -> p (a b)")
                nc.scalar.activation(ho_f[:], hop[:], AF.Silu, bias=ob1[:])
                o2p = egp.tile([64, NCOL], F32, tag="o2p")
                nc.tensor.matmul(o2p[:], ow2[:], ho_f[:], start=True, stop=True)
                outf = eg.tile([64, BL, NE], F32, tag="outf")
                outf_f = outf[:, :, :].rearrange("p a b -> p (a b)")
                nc.vector.tensor_scalar_add(outf_f[:], o2p[:], ob2[:])
                t80 = eg.tile([NE, BL, S], F32, tag="t80")
                for bl in range(BL):
                    tp = egp.tile([NE, S], F32, tag="tp")
                    nc.tensor.transpose(tp[:], outf[:, bl, :],
                                        id128[0:64, 0:64])
                    nc.scalar.copy(t80[:, bl, :], tp[:])
                    nc.gpsimd.indirect_dma_start(
                        out=out_d[:, :],
                        out_offset=bass.IndirectOffsetOnAxis(
                            ap=oxt[:, bl:bl + 1], axis=0),
                        in_=t80[:, bl, :],
                        in_offset=None,
                    )
    nc.compile()
    return nc


def _get_nc(T, TH):
    if (T, TH) not in _NC_CACHE:
        _NC_CACHE[(T, TH)] = build_nc(T, TH)
    return _NC_CACHE[(T, TH)]


def _prep(inputs):
    h = np.ascontiguousarray(np.asarray(inputs["h_flat"], dtype=np.float32))
    z = np.asarray(inputs["z_flat"]).astype(np.int64)
    ef = np.asarray(inputs["e_feat"], dtype=np.float32)
    pj = np.asarray(inputs["path_j"]).astype(np.int64)
    pk = np.asarray(inputs["path_k"]).astype(np.int64)
    r0j = np.asarray(inputs["path_r0j"], dtype=np.float32)
    r0k = np.asarray(inputs["path_r0k"], dtype=np.float32)
    rjk = np.asarray(inputs["path_rjk"], dtype=np.float32)
    cosa = np.asarray(inputs["path_cosangle"], dtype=np.float32)
    pb = np.asarray(inputs["path_batch"]).astype(np.int64)
    zemb = np.asarray(inputs["z_emb"], dtype=np.float32)
    assert int(inputs["bsz"]) == B

    order = np.argsort(pb, kind="stable")
    lows = [[] for _ in range(NCORES)]
    highs = [[] for _ in range(NCORES)]
    rr_lo = rr_hi = 0
    for b in range(B):
        idxs = order[pb[order] == b]
        nt = (len(idxs) + F - 1) // F
        if nt > NCORES:
            raise ValueError(f"batch {b} has {len(idxs)} paths > {NCORES*F}")
        for k in range(nt):
            til = (b, idxs[k * F:(k + 1) * F])
            if b < B // 2:
                lows[rr_lo % NCORES].append(til)
                rr_lo += 1
            else:
                highs[rr_hi % NCORES].append(til)
                rr_hi += 1
    TH = max(1, max(len(pc) for pc in lows))
    T = TH + max(1, max(len(pc) for pc in highs))
    percore = []
    for c in range(NCORES):
        pc = list(lows[c]) + [None] * (TH - len(lows[c])) + list(highs[c])
        percore.append(pc)
    W = T * F

    hT = h.T  # (128, 1024)
    ezT = zemb.T  # (32, NZ)
    in_maps = []
    for c in range(NCORES):
        hjT = np.zeros((ATOM, W), np.float32)
        hkT = np.zeros((ATOM, W), np.float32)
        ejk2 = np.zeros((128, W), np.float32)
        r96 = np.full((96, W), 10.0, np.float32)
        cos1 = np.zeros((1, W), np.float32)
        oidx = np.empty((BL, NE, 1), np.int32)
        gidx = np.empty((BL, 128, 1), np.int32)
        for bl in range(BL):
            oidx[bl, :, 0] = (BL * c + bl) * NE + np.arange(NE)
            gidx[bl, :, 0] = (BL * c + bl) * 128 + np.arange(128)
        sidx = np.empty((T, 128, 1), np.int32)
        sidx[:, :, 0] = 16 * 128 + np.arange(128)
        for t, til in enumerate(percore[c]):
            if til is None:
                continue
            b, idxs = til
            n = len(idxs)
            ts = slice(t * F, t * F + n)
            jj = pj[idxs]
            kk = pk[idxs]
            hjT[:, ts] = hT[:, jj]
            hkT[:, ts] = hT[:, kk]
            ejk2[0:32, ts] = ezT[:, z[jj]]
            ejk2[32:64, ts] = ezT[:, z[kk]]
            ejk2[64:96, ts] = ejk2[0:32, ts]
            ejk2[96:128, ts] = ejk2[32:64, ts]
            r96[0:32, ts] = r0j[idxs]
            r96[32:64, ts] = r0k[idxs]
            r96[64:96, ts] = rjk[idxs]
            cos1[0, ts] = cosa[idxs]
            sidx[t, :, 0] = b * 128 + np.arange(128)
        in_maps.append({
            "hjT": hjT, "hkT": hkT, "ejk2": ejk2, "r96": r96,
            "cos1": cos1, "oidx": oidx, "gidx": gidx, "sidx": sidx,
        })

    # params (replicated)
    gm_w1 = np.asarray(inputs["gm_w1"], np.float32)
    pe_w1 = np.asarray(inputs["pe_w1"], np.float32)
    pe_w2 = np.asarray(inputs["pe_w2"], np.float32)
    pe_w3 = np.asarray(inputs["pe_w3"], np.float32)
    w1ab = pe_w1[0:64, :]
    w1c = pe_w1[64:96, :]
    w1ab2 = np.zeros((128, 128), np.float32)
    w1ab2[0:64, 0:64] = w1ab
    w1ab2[64:128, 64:128] = w1ab
    w2bd = np.zeros((128, 128), np.float32)
    w2bd[0:64, 0:64] = pe_w2
    w2bd[64:128, 64:128] = pe_w2
    w3bd = np.zeros((128, 128), np.float32)
    w3bd[0:64, 0:64] = pe_w3
    w3bd[64:128, 64:128] = pe_w3
    pe_b1 = np.asarray(inputs["pe_b1"], np.float32)
    pe_b2 = np.asarray(inputs["pe_b2"], np.float32)
    pe_b3 = np.asarray(inputs["pe_b3"], np.float32)
    params = {
        "gw1a": np.ascontiguousarray(gm_w1[0:128, :]),
        "gw1b": np.ascontiguousarray(gm_w1[128:256, :]),
        "gw1c": np.ascontiguousarray(gm_w1[256:353, :]),
        "gb1": np.asarray(inputs["gm_b1"], np.float32)[:, None],
        "gw2": np.asarray(inputs["gm_w2"], np.float32),
        "gb2": np.asarray(inputs["gm_b2"], np.float32)[:, None],
        "gw3": np.asarray(inputs["gm_w3"], np.float32),
        "gb3": np.asarray(inputs["gm_b3"], np.float32)[:, None],
        "w1ab2": w1ab2,
        "b1c": pe_b1[:, None].astype(np.float32),
        "w2bd": w2bd,
        "b2c2": np.concatenate([pe_b2, pe_b2])[:, None].astype(np.float32),
        "w3bd": w3bd,
        "b3c2": np.concatenate([pe_b3, pe_b3])[:, None].astype(np.float32),
        "efev": np.ascontiguousarray(ef.T[:, 0::2]),
        "efod": np.ascontiguousarray(ef.T[:, 1::2]),
        "w1c": np.ascontiguousarray(w1c),
        "ow1": np.asarray(inputs["op_w1"], np.float32),
        "ob1": np.asarray(inputs["op_b1"], np.float32)[:, None],
        "ow2": np.asarray(inputs["op_w2"], np.float32),
        "ob2": np.asarray(inputs["op_b2"], np.float32)[:, None],
        "rb96": -np.tile(np.arange(RBF, dtype=np.float32), 3)[:, None],
        "ones128": np.ones((1, 128), np.float32),
        "id128": np.eye(128, dtype=np.float32),
    }
    for m in in_maps:
        m.update(params)
    return T, TH, in_maps


def _ensure_ntff_hook():
    """Inject antenv.axon_hooks (missing in this image) so trace=True works."""
    try:
        from antenv.axon_hooks import get_axon_ntff_profile_hook  # noqa: F401
        return
    except ImportError:
        pass
    import sys
    import types

    import antenv
    mod = types.ModuleType("antenv.axon_hooks")
    mod._hook = None
    mod.set_axon_ntff_profile_hook = lambda h: setattr(mod, "_hook", h)
    mod.get_axon_ntff_profile_hook = lambda: mod._hook
    sys.modules["antenv.axon_hooks"] = mod
    antenv.axon_hooks = mod
    try:
        from trn_agent_boot.trn_boot import _ntff_profile_via_ctypes
        mod._hook = _ntff_profile_via_ctypes("/opt/axon/libaxon_pjrt.so")
    except Exception as e:  # degrade to no-trace
        print("ntff hook setup failed:", e)


def kernel(**inputs) -> np.ndarray:
    T, TH, in_maps = _prep(inputs)
    nc = _get_nc(T, TH)
    trace = bool(int(os.environ.get("KERNEL_TRACE", "0")))
    if trace:
        _ensure_ntff_hook()
        import concourse.bass_utils as _bu
        _bu.upload_artifacts = lambda d: "local"
    res = run_bass_kernel_spmd(nc, in_maps, list(range(NCORES)), trace=trace,
                               tmpdir=os.environ.get("KERNEL_TRACE_DIR"))
    global LAST_RESULTS
    LAST_RESULTS = res
    out = np.empty((B, NE, S), np.float32)
    for c in range(NCORES):
        oc = np.asarray(res.results[c]["out"], np.float32).reshape(B, NE, S)
        out[BL * c:BL * c + BL] = oc[BL * c:BL * c + BL]
    return out


LAST_RESULTS = None
